# revision 12
# baseline (speedup 1.0000x reference)
"""Trainium2 Bass kernel for nn_LinearRTUs (rotation-unit layer).

Computes, for B=16384, D=H=1024:
    g    = r*cos(theta); phi = r*sin(theta); norm = sqrt(max(1-r^2, 0))
    h_t_c1 = g*h_c1 - phi*h_c2 + norm*(x @ w1)
    h_t_c2 = g*h_c2 + phi*h_c1 + norm*(x @ w2)
    h_t    = relu(concat(h_t_c1, h_t_c2))

Sharding: data-parallel over the batch dim across 8 NeuronCores
(2048 rows per core); w1/w2/r/theta replicated.

Per-core mapping:
  - norm is folded into the weights once (w' = w * norm), so the matmul
    psum directly accumulates norm*(x@w).
  - x tiles are transposed on the TensorEngine (fp32 transpose via
    identity matmul) so the contraction dim D lands on partitions.
  - matmuls run as float32r (full-speed fp32 mode, 1 cyc/row at N=512).
  - the rotation combine u = g*h1 - phi*h2 runs on the VectorEngine; it
    is added into the matmul psum with one extra identity matmul per
    accumulation group (PE does the add, saving DVE psum traffic).
  - ScalarEngine copies psum->sbuf (h_t_c1/h_t_c2) and applies relu for
    the concat output; stores issue from the ACT HWDGE ring, loads from
    the SP ring.
"""

import os

import numpy as np

B, D, H = 16384, 1024, 1024
NCORES = 8
BS = B // NCORES  # 2048 rows per core
P = 128
KT = D // P  # 8 contraction tiles
MT = BS // P  # 16 batch tiles per core
NSPLIT = 512  # psum accumulation-group width (fp32 bank limit)
NH = H // NSPLIT  # 2

_prog = None
last_result = None  # BassKernelResults of the most recent run (for test.py)


def _build():
    from contextlib import ExitStack

    import concourse.tile as tile
    from concourse import bacc, mybir
    from concourse.masks import make_identity

    f32 = mybir.dt.float32
    f32r = mybir.dt.float32r
    AF = mybir.ActivationFunctionType

    nc = bacc.Bacc(
        "TRN2",
        target_bir_lowering=False,
        debug=False,
        enable_asserts=True,
        num_devices=NCORES,
    )

    x_d = nc.dram_tensor("x_t", (BS, D), f32, kind="ExternalInput").ap()
    h1_d = nc.dram_tensor("h_c1", (BS, H), f32, kind="ExternalInput").ap()
    h2_d = nc.dram_tensor("h_c2", (BS, H), f32, kind="ExternalInput").ap()
    r_d = nc.dram_tensor("r_param", (1, H), f32, kind="ExternalInput").ap()
    th_d = nc.dram_tensor("theta_param", (1, H), f32, kind="ExternalInput").ap()
    w1_d = nc.dram_tensor("w1", (D, H), f32, kind="ExternalInput").ap()
    w2_d = nc.dram_tensor("w2", (D, H), f32, kind="ExternalInput").ap()
    o1_d = nc.dram_tensor("h_t_c1", (BS, H), f32, kind="ExternalOutput").ap()
    o2_d = nc.dram_tensor("h_t_c2", (BS, H), f32, kind="ExternalOutput").ap()
    ht_d = nc.dram_tensor("h_t", (BS, 2 * H), f32, kind="ExternalOutput").ap()

    with tile.TileContext(nc) as tc, ExitStack() as ctx:
        const = ctx.enter_context(tc.tile_pool(name="const", bufs=1))
        wres = ctx.enter_context(tc.tile_pool(name="wres", bufs=1))
        loads = ctx.enter_context(tc.tile_pool(name="loads", bufs=4))
        wload = ctx.enter_context(tc.tile_pool(name="wload", bufs=1))
        xtp = ctx.enter_context(tc.tile_pool(name="xtp", bufs=2))
        tmpv = ctx.enter_context(tc.tile_pool(name="tmpv", bufs=1))
        tmpu = ctx.enter_context(tc.tile_pool(name="tmpu", bufs=2))
        outp = ctx.enter_context(tc.tile_pool(name="outp", bufs=2))
        pmm = ctx.enter_context(tc.tile_pool(name="pmm", bufs=6, space="PSUM"))
        ptr = ctx.enter_context(tc.tile_pool(name="ptr", bufs=1, space="PSUM"))

        ident = const.tile([P, P], f32)
        make_identity(nc, ident[:])
        identr = const.tile([P, P], f32r)
        nc.vector.tensor_copy(identr[:], ident[:])

        # ---- params: broadcast r/theta across partitions, derive g/phi/norm
        g_b = const.tile([P, H], f32)
        phi_b = const.tile([P, H], f32)
        norm_b = const.tile([P, H], f32)

        rrow = tmpv.tile([1, H], f32, tag="v3")
        nc.sync.dma_start(out=rrow[:], in_=r_d[:, :])
        r_b = tmpv.tile([P, H], f32, tag="v1")
        nc.gpsimd.partition_broadcast(r_b[:], rrow[:])

        throw_ = tmpv.tile([1, H], f32, tag="v4")
        nc.sync.dma_start(out=throw_[:], in_=th_d[:, :])
        th_b = tmpv.tile([P, H], f32, tag="v2")
        nc.gpsimd.partition_broadcast(th_b[:], throw_[:])

        # ACT's Sin LUT is only valid on [-pi, pi]; theta is in [0, 2pi).
        # cos(th) = 1 - 2*sin^2(th/2), th/2 in [0, pi).
        # sin(th) = -sin(th - pi), th - pi in [-pi, pi); we keep
        # phin_b = r*sin(th-pi) = -phi and flip the signs in the combine.
        sh_b = tmpv.tile([P, H], f32, tag="v3")
        nc.scalar.activation(out=sh_b[:], in_=th_b[:], func=AF.Sin, scale=0.5)
        nc.scalar.square(sh_b[:], sh_b[:])
        nc.vector.tensor_scalar(
            out=sh_b[:],
            in0=sh_b[:],
            scalar1=-2.0,
            scalar2=1.0,
            op0=mybir.AluOpType.mult,
            op1=mybir.AluOpType.add,
        )
        nc.vector.tensor_mul(g_b[:], r_b[:], sh_b[:])

        negpi = const.tile([P, 1], f32)
        nc.gpsimd.memset(negpi[:], float(-np.pi))
        sin_b = tmpv.tile([P, H], f32, tag="v4")
        nc.scalar.activation(out=sin_b[:], in_=th_b[:], func=AF.Sin, bias=negpi[:])
        nc.vector.tensor_mul(phi_b[:], r_b[:], sin_b[:])  # phi_b = -phi

        sq_b = tmpv.tile([P, H], f32, tag="v3")
        nc.scalar.square(sq_b[:], r_b[:])
        nc.vector.tensor_scalar(
            out=sq_b[:],
            in0=sq_b[:],
            scalar1=-1.0,
            scalar2=1.0,
            op0=mybir.AluOpType.mult,
            op1=mybir.AluOpType.add,
        )
        nc.vector.tensor_scalar_max(sq_b[:], sq_b[:], 0.0)
        nc.scalar.sqrt(norm_b[:], sq_b[:])

        # ---- fold norm into the weights: w' = w * norm (broadcast over rows)
        w1s = wres.tile([P, KT * H], f32r)
        w2s = wres.tile([P, KT * H], f32r)
        for wd, ws in ((w1_d, w1s), (w2_d, w2s)):
            for k in range(KT):
                wt = wload.tile([P, H], f32, tag="wt")
                nc.scalar.dma_start(out=wt[:], in_=wd[k * P : (k + 1) * P, :])
                nc.vector.tensor_mul(ws[:, k * H : (k + 1) * H], wt[:], norm_b[:])

        # ---- main loop over 128-row batch tiles
        for m in range(MT):
            if True:
                ms = slice(m * P, (m + 1) * P)
                xm = loads.tile([P, D], f32, tag="xm")
                nc.sync.dma_start(out=xm[:], in_=x_d[ms, :])
                h1 = loads.tile([P, H], f32, tag="h1")
                nc.sync.dma_start(out=h1[:], in_=h1_d[ms, :])
                h2 = loads.tile([P, H], f32, tag="h2")
                nc.sync.dma_start(out=h2[:], in_=h2_d[ms, :])

                # transpose x subtile: [batch, D] -> [D, batch]
                px = ptr.tile([P, D], f32)
                for k in range(KT):
                    nc.tensor.transpose(
                        px[:, k * P : (k + 1) * P], xm[:, k * P : (k + 1) * P], ident[:]
                    )
                xT = xtp.tile([P, D], f32r)
                nc.scalar.copy(xT[:], px[:])

                # rotation combine on DVE (in-place accumulate into u tiles)
                u1 = tmpu.tile([P, H], f32r, tag="u1")
                nc.vector.tensor_mul(u1[:], h1[:], g_b[:])
                v2 = tmpv.tile([P, H], f32, tag="vv")
                nc.vector.tensor_mul(v2[:], h2[:], phi_b[:])
                nc.vector.tensor_add(u1[:], u1[:], v2[:])  # g*h1 + (-phi)*h2

                u2 = tmpu.tile([P, H], f32r, tag="u2")
                nc.vector.tensor_mul(u2[:], h2[:], g_b[:])
                v4 = tmpv.tile([P, H], f32, tag="vv")
                nc.vector.tensor_mul(v4[:], h1[:], phi_b[:])
                nc.vector.tensor_sub(u2[:], u2[:], v4[:])  # g*h2 - (-phi)*h1

                o1 = outp.tile([P, H], f32, tag="o1")
                o2 = outp.tile([P, H], f32, tag="o2")
                rht = outp.tile([P, 2 * H], f32, tag="rht")

                for wi, (ws, u, o) in enumerate(((w1s, u1, o1), (w2s, u2, o2))):
                    for n in range(NH):
                        ps = pmm.tile([P, NSPLIT], f32, tag="ps")
                        ns = slice(n * NSPLIT, (n + 1) * NSPLIT)
                        for k in range(KT):
                            nc.tensor.matmul(
                                ps[:],
                                xT[:, k * P : (k + 1) * P],
                                ws[:, k * H + n * NSPLIT : k * H + (n + 1) * NSPLIT],
                                start=(k == 0),
                                stop=False,
                            )
                        # psum += u  (identity matmul: I.T @ u = u)
                        nc.tensor.matmul(
                            ps[:],
                            identr[:],
                            u[:, ns],
                            start=False,
                            stop=True,
                        )
                        nc.scalar.copy(o[:, ns], ps[:])
                        nc.scalar.activation(
                            out=rht[:, wi * H + n * NSPLIT : wi * H + (n + 1) * NSPLIT],
                            in_=ps[:],
                            func=AF.Relu,
                        )

                nc.scalar.dma_start(out=o1_d[ms, :], in_=o1[:])
                nc.scalar.dma_start(out=o2_d[ms, :], in_=o2[:])
                nc.scalar.dma_start(out=ht_d[ms, :], in_=rht[:])

    nc.compile()
    return nc


def _get_prog():
    global _prog
    if _prog is None:
        _prog = _build()
    return _prog


def _ensure_axon_ntff_hook():
    """Dev-only: register the NTFF profile hook that this image's antenv
    package lacks, so trace=True yields exec_time_ns. Used only when
    BASS_KERNEL_TRACE=1 (never in the grading path)."""
    import sys
    import types

    try:
        import antenv.axon_hooks  # noqa: F401

        return
    except ImportError:
        pass
    hook = None
    try:
        from trn_agent_boot.trn_boot import _ntff_profile_via_ctypes

        hook = _ntff_profile_via_ctypes("/opt/axon/libaxon_pjrt.so")
    except Exception:
        hook = None
    mod = types.ModuleType("antenv.axon_hooks")
    mod.get_axon_ntff_profile_hook = lambda: hook
    mod.set_axon_ntff_profile_hook = lambda h: None
    sys.modules["antenv.axon_hooks"] = mod


def kernel(**inputs):
    global last_result
    from concourse import bass_utils

    nc = _get_prog()

    arrs = {k: np.ascontiguousarray(np.asarray(v), dtype=np.float32) for k, v in inputs.items()}

    in_maps = []
    for c in range(NCORES):
        sl = slice(c * BS, (c + 1) * BS)
        in_maps.append(
            {
                "x_t": np.ascontiguousarray(arrs["x_t"][sl]),
                "h_c1": np.ascontiguousarray(arrs["h_c1"][sl]),
                "h_c2": np.ascontiguousarray(arrs["h_c2"][sl]),
                "r_param": arrs["r_param"],
                "theta_param": arrs["theta_param"],
                "w1": arrs["w1"],
                "w2": arrs["w2"],
            }
        )

    trace = os.environ.get("BASS_KERNEL_TRACE") == "1"
    kwargs = {}
    if trace:
        _ensure_axon_ntff_hook()
        # keep artifacts local; no bucket upload from this container
        bass_utils.upload_artifacts = lambda tmpdir: tmpdir
        tdir = os.environ.get("BASS_KERNEL_TRACE_DIR")
        if tdir:
            os.makedirs(tdir, exist_ok=True)
            kwargs["tmpdir"] = tdir
    try:
        res = bass_utils.run_bass_kernel_spmd(
            nc, in_maps, core_ids=list(range(NCORES)), trace=trace, **kwargs
        )
    except Exception:
        if not trace:
            raise
        import traceback

        traceback.print_exc()
        print("trace path failed; retrying without trace", flush=True)
        res = bass_utils.run_bass_kernel_spmd(
            nc, in_maps, core_ids=list(range(NCORES)), trace=False
        )
    last_result = res
    outs = res.results

    h_t_c1 = np.concatenate([outs[c]["h_t_c1"] for c in range(NCORES)], axis=0)
    h_t_c2 = np.concatenate([outs[c]["h_t_c2"] for c in range(NCORES)], axis=0)
    h_t = np.concatenate([outs[c]["h_t"] for c in range(NCORES)], axis=0)
    return (h_t_c1, h_t_c2, h_t)


# revision 13
# speedup vs baseline: 1.1312x; 1.1312x over previous
"""Trainium2 Bass kernel for nn_LinearRTUs (rotation-unit layer).

Computes, for B=16384, D=H=1024:
    g    = r*cos(theta); phi = r*sin(theta); norm = sqrt(max(1-r^2, 0))
    h_t_c1 = g*h_c1 - phi*h_c2 + norm*(x @ w1)
    h_t_c2 = g*h_c2 + phi*h_c1 + norm*(x @ w2)
    h_t    = relu(concat(h_t_c1, h_t_c2))

Sharding: data-parallel over the batch dim across 8 NeuronCores
(2048 rows per core); w1/w2/r/theta replicated.

Per-core mapping:
  - norm is folded into the weights once (w' = w * norm), so the matmul
    psum directly accumulates norm*(x@w).
  - x tiles are transposed on the TensorEngine (fp32 transpose via
    identity matmul) so the contraction dim D lands on partitions.
  - matmuls run as float32r (full-speed fp32 mode, 1 cyc/row at N=512).
  - the rotation combine u = g*h1 - phi*h2 runs on the VectorEngine; it
    is added into the matmul psum with one extra identity matmul per
    accumulation group (PE does the add, saving DVE psum traffic).
  - ScalarEngine copies psum->sbuf (h_t_c1/h_t_c2) and applies relu for
    the concat output; stores issue from the ACT HWDGE ring, loads from
    the SP ring.
"""

import os

import numpy as np

B, D, H = 16384, 1024, 1024
NCORES = 8
BS = B // NCORES  # 2048 rows per core
P = 128
KT = D // P  # 8 contraction tiles
MT = BS // P  # 16 batch tiles per core
NSPLIT = 512  # psum accumulation-group width (fp32 bank limit)
NH = H // NSPLIT  # 2

_prog = None
last_result = None  # BassKernelResults of the most recent run (for test.py)


def _build():
    from contextlib import ExitStack

    import concourse.tile as tile
    from concourse import bacc, mybir
    from concourse.masks import make_identity

    f32 = mybir.dt.float32
    f32r = mybir.dt.float32r
    AF = mybir.ActivationFunctionType

    nc = bacc.Bacc(
        "TRN2",
        target_bir_lowering=False,
        debug=False,
        enable_asserts=True,
        num_devices=NCORES,
    )

    x_d = nc.dram_tensor("x_t", (BS, D), f32, kind="ExternalInput").ap()
    h1_d = nc.dram_tensor("h_c1", (BS, H), f32, kind="ExternalInput").ap()
    h2_d = nc.dram_tensor("h_c2", (BS, H), f32, kind="ExternalInput").ap()
    r_d = nc.dram_tensor("r_param", (1, H), f32, kind="ExternalInput").ap()
    th_d = nc.dram_tensor("theta_param", (1, H), f32, kind="ExternalInput").ap()
    w1_d = nc.dram_tensor("w1", (D, H), f32, kind="ExternalInput").ap()
    w2_d = nc.dram_tensor("w2", (D, H), f32, kind="ExternalInput").ap()
    o1_d = nc.dram_tensor("h_t_c1", (BS, H), f32, kind="ExternalOutput").ap()
    o2_d = nc.dram_tensor("h_t_c2", (BS, H), f32, kind="ExternalOutput").ap()
    ht_d = nc.dram_tensor("h_t", (BS, 2 * H), f32, kind="ExternalOutput").ap()

    with tile.TileContext(nc) as tc, ExitStack() as ctx:
        const = ctx.enter_context(tc.tile_pool(name="const", bufs=1))
        wres = ctx.enter_context(tc.tile_pool(name="wres", bufs=1))
        loads = ctx.enter_context(tc.tile_pool(name="loads", bufs=4))
        wload = ctx.enter_context(tc.tile_pool(name="wload", bufs=2))
        xtp = ctx.enter_context(tc.tile_pool(name="xtp", bufs=2))
        tmpv = ctx.enter_context(tc.tile_pool(name="tmpv", bufs=1))
        tmpu = ctx.enter_context(tc.tile_pool(name="tmpu", bufs=2))
        outp = ctx.enter_context(tc.tile_pool(name="outp", bufs=2))
        pmm = ctx.enter_context(tc.tile_pool(name="pmm", bufs=6, space="PSUM"))
        ptr = ctx.enter_context(tc.tile_pool(name="ptr", bufs=1, space="PSUM"))

        ident = const.tile([P, P], f32)
        make_identity(nc, ident[:])
        identr = const.tile([P, P], f32r)
        nc.vector.tensor_copy(identr[:], ident[:])

        # ---- params: broadcast r/theta across partitions, derive g/phi/norm
        g_b = const.tile([P, H], f32)
        phi_b = const.tile([P, H], f32)
        norm_b = const.tile([P, H], f32)

        rrow = tmpv.tile([1, H], f32, tag="v3")
        nc.sync.dma_start(out=rrow[:], in_=r_d[:, :])
        r_b = tmpv.tile([P, H], f32, tag="v1")
        nc.gpsimd.partition_broadcast(r_b[:], rrow[:])

        throw_ = tmpv.tile([1, H], f32, tag="v4")
        nc.sync.dma_start(out=throw_[:], in_=th_d[:, :])
        th_b = tmpv.tile([P, H], f32, tag="v2")
        nc.gpsimd.partition_broadcast(th_b[:], throw_[:])

        # ACT's Sin LUT is only valid on [-pi, pi]; theta is in [0, 2pi).
        # cos(th) = 1 - 2*sin^2(th/2), th/2 in [0, pi).
        # sin(th) = -sin(th - pi), th - pi in [-pi, pi); we keep
        # phin_b = r*sin(th-pi) = -phi and flip the signs in the combine.
        sh_b = tmpv.tile([P, H], f32, tag="v3")
        nc.scalar.activation(out=sh_b[:], in_=th_b[:], func=AF.Sin, scale=0.5)
        nc.scalar.square(sh_b[:], sh_b[:])
        nc.vector.tensor_scalar(
            out=sh_b[:],
            in0=sh_b[:],
            scalar1=-2.0,
            scalar2=1.0,
            op0=mybir.AluOpType.mult,
            op1=mybir.AluOpType.add,
        )
        nc.vector.tensor_mul(g_b[:], r_b[:], sh_b[:])

        negpi = const.tile([P, 1], f32)
        nc.gpsimd.memset(negpi[:], float(-np.pi))
        sin_b = tmpv.tile([P, H], f32, tag="v4")
        nc.scalar.activation(out=sin_b[:], in_=th_b[:], func=AF.Sin, bias=negpi[:])
        nc.vector.tensor_mul(phi_b[:], r_b[:], sin_b[:])  # phi_b = -phi

        sq_b = tmpv.tile([P, H], f32, tag="v3")
        nc.scalar.square(sq_b[:], r_b[:])
        nc.vector.tensor_scalar(
            out=sq_b[:],
            in0=sq_b[:],
            scalar1=-1.0,
            scalar2=1.0,
            op0=mybir.AluOpType.mult,
            op1=mybir.AluOpType.add,
        )
        nc.vector.tensor_scalar_max(sq_b[:], sq_b[:], 0.0)
        nc.scalar.sqrt(norm_b[:], sq_b[:])

        # ---- fold norm into the weights: w' = w * norm (broadcast over rows)
        w1s = wres.tile([P, KT * H], f32r)
        w2s = wres.tile([P, KT * H], f32r)
        for wd, ws in ((w1_d, w1s), (w2_d, w2s)):
            for k in range(KT):
                wt = wload.tile([P, H], f32, tag="wt")
                nc.sync.dma_start(out=wt[:], in_=wd[k * P : (k + 1) * P, :])
                nc.vector.tensor_mul(ws[:, k * H : (k + 1) * H], wt[:], norm_b[:])

        # ---- main loop over 128-row batch tiles
        for m in range(MT):
            if True:
                ms = slice(m * P, (m + 1) * P)
                xm = loads.tile([P, D], f32, tag="xm")
                nc.sync.dma_start(out=xm[:], in_=x_d[ms, :])
                h1 = loads.tile([P, H], f32, tag="h1")
                nc.sync.dma_start(out=h1[:], in_=h1_d[ms, :])
                h2 = loads.tile([P, H], f32, tag="h2")
                nc.sync.dma_start(out=h2[:], in_=h2_d[ms, :])

                # transpose x subtile: [batch, D] -> [D, batch]
                px = ptr.tile([P, D], f32)
                for k in range(KT):
                    nc.tensor.transpose(
                        px[:, k * P : (k + 1) * P], xm[:, k * P : (k + 1) * P], ident[:]
                    )
                xT = xtp.tile([P, D], f32r)
                nc.scalar.copy(xT[:], px[:])

                # rotation combine on DVE
                v1 = tmpv.tile([P, H], f32, tag="v1")
                nc.vector.tensor_mul(v1[:], h1[:], g_b[:])
                v2 = tmpv.tile([P, H], f32, tag="v2")
                nc.vector.tensor_mul(v2[:], h2[:], phi_b[:])
                u1 = tmpu.tile([P, H], f32r, tag="u1")
                nc.vector.tensor_add(u1[:], v1[:], v2[:])  # g*h1 + (-phi)*h2

                v3 = tmpv.tile([P, H], f32, tag="v3")
                nc.vector.tensor_mul(v3[:], h2[:], g_b[:])
                v4 = tmpv.tile([P, H], f32, tag="v4")
                nc.vector.tensor_mul(v4[:], h1[:], phi_b[:])
                u2 = tmpu.tile([P, H], f32r, tag="u2")
                nc.vector.tensor_sub(u2[:], v3[:], v4[:])  # g*h2 - (-phi)*h1

                o1 = outp.tile([P, H], f32, tag="o1")
                o2 = outp.tile([P, H], f32, tag="o2")
                r1 = outp.tile([P, H], f32, tag="r1")
                r2 = outp.tile([P, H], f32, tag="r2")

                for ws, u, o, r in ((w1s, u1, o1, r1), (w2s, u2, o2, r2)):
                    for n in range(NH):
                        ps = pmm.tile([P, NSPLIT], f32, tag="ps")
                        ns = slice(n * NSPLIT, (n + 1) * NSPLIT)
                        for k in range(KT):
                            nc.tensor.matmul(
                                ps[:],
                                xT[:, k * P : (k + 1) * P],
                                ws[:, k * H + n * NSPLIT : k * H + (n + 1) * NSPLIT],
                                start=(k == 0),
                                stop=False,
                            )
                        # psum += u  (identity matmul: I.T @ u = u)
                        nc.tensor.matmul(
                            ps[:],
                            identr[:],
                            u[:, ns],
                            start=False,
                            stop=True,
                        )
                        nc.scalar.copy(o[:, ns], ps[:])
                        nc.scalar.activation(out=r[:, ns], in_=ps[:], func=AF.Relu)

                nc.scalar.dma_start(out=o1_d[ms, :], in_=o1[:])
                nc.scalar.dma_start(out=o2_d[ms, :], in_=o2[:])
                nc.scalar.dma_start(out=ht_d[ms, 0:H], in_=r1[:])
                nc.scalar.dma_start(out=ht_d[ms, H : 2 * H], in_=r2[:])

    nc.compile()
    return nc


def _get_prog():
    global _prog
    if _prog is None:
        _prog = _build()
    return _prog


def _ensure_axon_ntff_hook():
    """Dev-only: register the NTFF profile hook that this image's antenv
    package lacks, so trace=True yields exec_time_ns. Used only when
    BASS_KERNEL_TRACE=1 (never in the grading path)."""
    import sys
    import types

    try:
        import antenv.axon_hooks  # noqa: F401

        return
    except ImportError:
        pass
    hook = None
    try:
        from trn_agent_boot.trn_boot import _ntff_profile_via_ctypes

        hook = _ntff_profile_via_ctypes("/opt/axon/libaxon_pjrt.so")
    except Exception:
        hook = None
    mod = types.ModuleType("antenv.axon_hooks")
    mod.get_axon_ntff_profile_hook = lambda: hook
    mod.set_axon_ntff_profile_hook = lambda h: None
    sys.modules["antenv.axon_hooks"] = mod


def kernel(**inputs):
    global last_result
    from concourse import bass_utils

    nc = _get_prog()

    arrs = {k: np.ascontiguousarray(np.asarray(v), dtype=np.float32) for k, v in inputs.items()}

    in_maps = []
    for c in range(NCORES):
        sl = slice(c * BS, (c + 1) * BS)
        in_maps.append(
            {
                "x_t": np.ascontiguousarray(arrs["x_t"][sl]),
                "h_c1": np.ascontiguousarray(arrs["h_c1"][sl]),
                "h_c2": np.ascontiguousarray(arrs["h_c2"][sl]),
                "r_param": arrs["r_param"],
                "theta_param": arrs["theta_param"],
                "w1": arrs["w1"],
                "w2": arrs["w2"],
            }
        )

    trace = os.environ.get("BASS_KERNEL_TRACE") == "1"
    kwargs = {}
    if trace:
        _ensure_axon_ntff_hook()
        # keep artifacts local; no bucket upload from this container
        bass_utils.upload_artifacts = lambda tmpdir: tmpdir
        tdir = os.environ.get("BASS_KERNEL_TRACE_DIR")
        if tdir:
            os.makedirs(tdir, exist_ok=True)
            kwargs["tmpdir"] = tdir
    try:
        res = bass_utils.run_bass_kernel_spmd(
            nc, in_maps, core_ids=list(range(NCORES)), trace=trace, **kwargs
        )
    except Exception:
        if not trace:
            raise
        import traceback

        traceback.print_exc()
        print("trace path failed; retrying without trace", flush=True)
        res = bass_utils.run_bass_kernel_spmd(
            nc, in_maps, core_ids=list(range(NCORES)), trace=False
        )
    last_result = res
    outs = res.results

    h_t_c1 = np.concatenate([outs[c]["h_t_c1"] for c in range(NCORES)], axis=0)
    h_t_c2 = np.concatenate([outs[c]["h_t_c2"] for c in range(NCORES)], axis=0)
    h_t = np.concatenate([outs[c]["h_t"] for c in range(NCORES)], axis=0)
    return (h_t_c1, h_t_c2, h_t)


# revision 14
# speedup vs baseline: 1.1750x; 1.0387x over previous
"""Trainium2 Bass kernel for nn_LinearRTUs (rotation-unit layer).

Computes, for B=16384, D=H=1024:
    g    = r*cos(theta); phi = r*sin(theta); norm = sqrt(max(1-r^2, 0))
    h_t_c1 = g*h_c1 - phi*h_c2 + norm*(x @ w1)
    h_t_c2 = g*h_c2 + phi*h_c1 + norm*(x @ w2)
    h_t    = relu(concat(h_t_c1, h_t_c2))

Sharding: data-parallel over the batch dim across 8 NeuronCores
(2048 rows per core); w1/w2/r/theta replicated.

Per-core mapping:
  - norm is folded into the weights once (w' = w * norm), so the matmul
    psum directly accumulates norm*(x@w).
  - x tiles are transposed on the TensorEngine (fp32 transpose via
    identity matmul) so the contraction dim D lands on partitions.
  - matmuls run as float32r (full-speed fp32 mode, 1 cyc/row at N=512).
  - the rotation combine u = g*h1 - phi*h2 runs on the VectorEngine; it
    is added into the matmul psum with one extra identity matmul per
    accumulation group (PE does the add, saving DVE psum traffic).
  - ScalarEngine copies psum->sbuf (h_t_c1/h_t_c2) and applies relu for
    the concat output; stores issue from the ACT HWDGE ring, loads from
    the SP ring.
"""

import os

import numpy as np

B, D, H = 16384, 1024, 1024
NCORES = 8
BS = B // NCORES  # 2048 rows per core
P = 128
KT = D // P  # 8 contraction tiles
MT = BS // P  # 16 batch tiles per core
NSPLIT = 512  # psum accumulation-group width (fp32 bank limit)
NH = H // NSPLIT  # 2

_prog = None
last_result = None  # BassKernelResults of the most recent run (for test.py)


def _build():
    from contextlib import ExitStack

    import concourse.tile as tile
    from concourse import bacc, mybir
    from concourse.masks import make_identity

    f32 = mybir.dt.float32
    f32r = mybir.dt.float32r
    AF = mybir.ActivationFunctionType

    nc = bacc.Bacc(
        "TRN2",
        target_bir_lowering=False,
        debug=False,
        enable_asserts=True,
        num_devices=NCORES,
    )

    x_d = nc.dram_tensor("x_t", (BS, D), f32, kind="ExternalInput").ap()
    h1_d = nc.dram_tensor("h_c1", (BS, H), f32, kind="ExternalInput").ap()
    h2_d = nc.dram_tensor("h_c2", (BS, H), f32, kind="ExternalInput").ap()
    r_d = nc.dram_tensor("r_param", (1, H), f32, kind="ExternalInput").ap()
    th_d = nc.dram_tensor("theta_param", (1, H), f32, kind="ExternalInput").ap()
    w1_d = nc.dram_tensor("w1", (D, H), f32, kind="ExternalInput").ap()
    w2_d = nc.dram_tensor("w2", (D, H), f32, kind="ExternalInput").ap()
    o1_d = nc.dram_tensor("h_t_c1", (BS, H), f32, kind="ExternalOutput").ap()
    o2_d = nc.dram_tensor("h_t_c2", (BS, H), f32, kind="ExternalOutput").ap()
    ht_d = nc.dram_tensor("h_t", (BS, 2 * H), f32, kind="ExternalOutput").ap()

    with tile.TileContext(nc) as tc, ExitStack() as ctx:
        const = ctx.enter_context(tc.tile_pool(name="const", bufs=1))
        wres = ctx.enter_context(tc.tile_pool(name="wres", bufs=1))
        loads = ctx.enter_context(tc.tile_pool(name="loads", bufs=4))
        wload = ctx.enter_context(tc.tile_pool(name="wload", bufs=2))
        xtp = ctx.enter_context(tc.tile_pool(name="xtp", bufs=2))
        tmpv = ctx.enter_context(tc.tile_pool(name="tmpv", bufs=1))
        tmpu = ctx.enter_context(tc.tile_pool(name="tmpu", bufs=2))
        outp = ctx.enter_context(tc.tile_pool(name="outp", bufs=2))
        pmm = ctx.enter_context(tc.tile_pool(name="pmm", bufs=6, space="PSUM"))
        ptr = ctx.enter_context(tc.tile_pool(name="ptr", bufs=1, space="PSUM"))

        ident = const.tile([P, P], f32)
        make_identity(nc, ident[:])
        identr = const.tile([P, P], f32r)
        nc.vector.tensor_copy(identr[:], ident[:])

        # ---- params: broadcast r/theta across partitions, derive g/phi/norm
        g_b = const.tile([P, H], f32)
        phi_b = const.tile([P, H], f32)
        norm_b = const.tile([P, H], f32)

        rrow = tmpv.tile([1, H], f32, tag="v3")
        nc.sync.dma_start(out=rrow[:], in_=r_d[:, :])
        r_b = tmpv.tile([P, H], f32, tag="v1")
        nc.gpsimd.partition_broadcast(r_b[:], rrow[:])

        throw_ = tmpv.tile([1, H], f32, tag="v4")
        nc.sync.dma_start(out=throw_[:], in_=th_d[:, :])
        th_b = tmpv.tile([P, H], f32, tag="v2")
        nc.gpsimd.partition_broadcast(th_b[:], throw_[:])

        # ACT's Sin LUT is only valid on [-pi, pi]; theta is in [0, 2pi).
        # cos(th) = 1 - 2*sin^2(th/2), th/2 in [0, pi).
        # sin(th) = -sin(th - pi), th - pi in [-pi, pi); we keep
        # phin_b = r*sin(th-pi) = -phi and flip the signs in the combine.
        sh_b = tmpv.tile([P, H], f32, tag="v3")
        nc.scalar.activation(out=sh_b[:], in_=th_b[:], func=AF.Sin, scale=0.5)
        nc.scalar.square(sh_b[:], sh_b[:])
        nc.vector.tensor_scalar(
            out=sh_b[:],
            in0=sh_b[:],
            scalar1=-2.0,
            scalar2=1.0,
            op0=mybir.AluOpType.mult,
            op1=mybir.AluOpType.add,
        )
        nc.vector.tensor_mul(g_b[:], r_b[:], sh_b[:])

        negpi = const.tile([P, 1], f32)
        nc.gpsimd.memset(negpi[:], float(-np.pi))
        sin_b = tmpv.tile([P, H], f32, tag="v4")
        nc.scalar.activation(out=sin_b[:], in_=th_b[:], func=AF.Sin, bias=negpi[:])
        nc.vector.tensor_mul(phi_b[:], r_b[:], sin_b[:])  # phi_b = -phi

        sq_b = tmpv.tile([P, H], f32, tag="v3")
        nc.scalar.square(sq_b[:], r_b[:])
        nc.vector.tensor_scalar(
            out=sq_b[:],
            in0=sq_b[:],
            scalar1=-1.0,
            scalar2=1.0,
            op0=mybir.AluOpType.mult,
            op1=mybir.AluOpType.add,
        )
        nc.vector.tensor_scalar_max(sq_b[:], sq_b[:], 0.0)
        nc.scalar.sqrt(norm_b[:], sq_b[:])

        # first batch tile's loads go ahead of the 8MB weight preload on
        # the SP HWDGE ring so transposes/combines start immediately
        pre = {}
        for m in range(2):
            ms = slice(m * P, (m + 1) * P)
            xm = loads.tile([P, D], f32, tag="xm")
            nc.sync.dma_start(out=xm[:], in_=x_d[ms, :])
            h1 = loads.tile([P, H], f32, tag="h1")
            nc.sync.dma_start(out=h1[:], in_=h1_d[ms, :])
            h2 = loads.tile([P, H], f32, tag="h2")
            nc.sync.dma_start(out=h2[:], in_=h2_d[ms, :])
            pre[m] = (xm, h1, h2)

        # ---- fold norm into the weights: w' = w * norm (broadcast over rows)
        w1s = wres.tile([P, KT * H], f32r)
        w2s = wres.tile([P, KT * H], f32r)
        for wd, ws in ((w1_d, w1s), (w2_d, w2s)):
            for k in range(KT):
                wt = wload.tile([P, H], f32, tag="wt")
                nc.sync.dma_start(out=wt[:], in_=wd[k * P : (k + 1) * P, :])
                nc.vector.tensor_mul(ws[:, k * H : (k + 1) * H], wt[:], norm_b[:])

        # ---- main loop over 128-row batch tiles
        for m in range(MT):
            if True:
                ms = slice(m * P, (m + 1) * P)
                if m in pre:
                    xm, h1, h2 = pre[m]
                else:
                    xm = loads.tile([P, D], f32, tag="xm")
                    nc.sync.dma_start(out=xm[:], in_=x_d[ms, :])
                    h1 = loads.tile([P, H], f32, tag="h1")
                    nc.sync.dma_start(out=h1[:], in_=h1_d[ms, :])
                    h2 = loads.tile([P, H], f32, tag="h2")
                    nc.sync.dma_start(out=h2[:], in_=h2_d[ms, :])

                # transpose x subtile: [batch, D] -> [D, batch]
                px = ptr.tile([P, D], f32)
                for k in range(KT):
                    nc.tensor.transpose(
                        px[:, k * P : (k + 1) * P], xm[:, k * P : (k + 1) * P], ident[:]
                    )
                xT = xtp.tile([P, D], f32r)
                nc.scalar.copy(xT[:], px[:])

                # rotation combine on DVE
                v1 = tmpv.tile([P, H], f32, tag="v1")
                nc.vector.tensor_mul(v1[:], h1[:], g_b[:])
                v2 = tmpv.tile([P, H], f32, tag="v2")
                nc.vector.tensor_mul(v2[:], h2[:], phi_b[:])
                u1 = tmpu.tile([P, H], f32r, tag="u1")
                nc.vector.tensor_add(u1[:], v1[:], v2[:])  # g*h1 + (-phi)*h2

                v3 = tmpv.tile([P, H], f32, tag="v3")
                nc.vector.tensor_mul(v3[:], h2[:], g_b[:])
                v4 = tmpv.tile([P, H], f32, tag="v4")
                nc.vector.tensor_mul(v4[:], h1[:], phi_b[:])
                u2 = tmpu.tile([P, H], f32r, tag="u2")
                nc.vector.tensor_sub(u2[:], v3[:], v4[:])  # g*h2 - (-phi)*h1

                o1 = outp.tile([P, H], f32, tag="o1")
                o2 = outp.tile([P, H], f32, tag="o2")
                r1 = outp.tile([P, H], f32, tag="r1")
                r2 = outp.tile([P, H], f32, tag="r2")

                for ws, u, o, r in ((w1s, u1, o1, r1), (w2s, u2, o2, r2)):
                    for n in range(NH):
                        ps = pmm.tile([P, NSPLIT], f32, tag="ps")
                        ns = slice(n * NSPLIT, (n + 1) * NSPLIT)
                        for k in range(KT):
                            nc.tensor.matmul(
                                ps[:],
                                xT[:, k * P : (k + 1) * P],
                                ws[:, k * H + n * NSPLIT : k * H + (n + 1) * NSPLIT],
                                start=(k == 0),
                                stop=False,
                            )
                        # psum += u  (identity matmul: I.T @ u = u)
                        nc.tensor.matmul(
                            ps[:],
                            identr[:],
                            u[:, ns],
                            start=False,
                            stop=True,
                        )
                        nc.scalar.copy(o[:, ns], ps[:])
                        nc.scalar.activation(out=r[:, ns], in_=ps[:], func=AF.Relu)

                nc.scalar.dma_start(out=o1_d[ms, :], in_=o1[:])
                nc.scalar.dma_start(out=o2_d[ms, :], in_=o2[:])
                nc.scalar.dma_start(out=ht_d[ms, 0:H], in_=r1[:])
                nc.scalar.dma_start(out=ht_d[ms, H : 2 * H], in_=r2[:])

    nc.compile()
    return nc


def _get_prog():
    global _prog
    if _prog is None:
        _prog = _build()
    return _prog


def _ensure_axon_ntff_hook():
    """Dev-only: register the NTFF profile hook that this image's antenv
    package lacks, so trace=True yields exec_time_ns. Used only when
    BASS_KERNEL_TRACE=1 (never in the grading path)."""
    import sys
    import types

    try:
        import antenv.axon_hooks  # noqa: F401

        return
    except ImportError:
        pass
    hook = None
    try:
        from trn_agent_boot.trn_boot import _ntff_profile_via_ctypes

        hook = _ntff_profile_via_ctypes("/opt/axon/libaxon_pjrt.so")
    except Exception:
        hook = None
    mod = types.ModuleType("antenv.axon_hooks")
    mod.get_axon_ntff_profile_hook = lambda: hook
    mod.set_axon_ntff_profile_hook = lambda h: None
    sys.modules["antenv.axon_hooks"] = mod


def kernel(**inputs):
    global last_result
    from concourse import bass_utils

    nc = _get_prog()

    arrs = {k: np.ascontiguousarray(np.asarray(v), dtype=np.float32) for k, v in inputs.items()}

    in_maps = []
    for c in range(NCORES):
        sl = slice(c * BS, (c + 1) * BS)
        in_maps.append(
            {
                "x_t": np.ascontiguousarray(arrs["x_t"][sl]),
                "h_c1": np.ascontiguousarray(arrs["h_c1"][sl]),
                "h_c2": np.ascontiguousarray(arrs["h_c2"][sl]),
                "r_param": arrs["r_param"],
                "theta_param": arrs["theta_param"],
                "w1": arrs["w1"],
                "w2": arrs["w2"],
            }
        )

    trace = os.environ.get("BASS_KERNEL_TRACE") == "1"
    kwargs = {}
    if trace:
        _ensure_axon_ntff_hook()
        # keep artifacts local; no bucket upload from this container
        bass_utils.upload_artifacts = lambda tmpdir: tmpdir
        tdir = os.environ.get("BASS_KERNEL_TRACE_DIR")
        if tdir:
            os.makedirs(tdir, exist_ok=True)
            kwargs["tmpdir"] = tdir
    try:
        res = bass_utils.run_bass_kernel_spmd(
            nc, in_maps, core_ids=list(range(NCORES)), trace=trace, **kwargs
        )
    except Exception:
        if not trace:
            raise
        import traceback

        traceback.print_exc()
        print("trace path failed; retrying without trace", flush=True)
        res = bass_utils.run_bass_kernel_spmd(
            nc, in_maps, core_ids=list(range(NCORES)), trace=False
        )
    last_result = res
    outs = res.results

    h_t_c1 = np.concatenate([outs[c]["h_t_c1"] for c in range(NCORES)], axis=0)
    h_t_c2 = np.concatenate([outs[c]["h_t_c2"] for c in range(NCORES)], axis=0)
    h_t = np.concatenate([outs[c]["h_t"] for c in range(NCORES)], axis=0)
    return (h_t_c1, h_t_c2, h_t)
